# revision 8
# baseline (speedup 1.0000x reference)
"""Trainium2 Bass kernel for the axial-attention block (nn_BCAM_49495203119370).

Self-contained: hardcodes shapes B=8, C=192, H=W=128, heads=8.
Shards batch across 8 NeuronCores (1 image per core).

Math per image (reference.py):
  out1 = Wp@x1+b ; out2 = Wp@x2+b  (1x1 conv)
  h-attention per head: q=h-tokens(out2), k=v=h-tokens(out1), l2-normalized,
    logits = q.k/0.01 (+bias, softmax-invariant, dropped), P=softmax, out3 = P@v + q
  w-attention per head: q=w-tokens(out1), k=v=w-tokens(out2) ...  out4 = P@v + q
  fusion = g*out3 + (1-g)*out4,  g = sigmoid(gate)
  out = conv(gelu(conv(conv(fusion,Wp,b),Wm1,bm1)),Wm2,bm2) + x1 + x2
"""
import sys

for _p in ("/opt/trn_rl_repo", "/root/.axon_site/_ro/trn_rl_repo"):
    if _p not in sys.path:
        sys.path.insert(0, _p)

import numpy as np

import concourse.bass as bass
import concourse.tile as tile
from concourse import bacc, mybir
from concourse.bass_utils import run_bass_kernel_spmd
from concourse.masks import make_identity

F32 = mybir.dt.float32
F32R = mybir.dt.float32r

C, H, W = 192, 128, 128
HEADS, CH = 8, 24           # channels per head
HW = H * W
NCH = 512                   # conv spatial chunk (= 4 h-rows)
NCHUNKS = HW // NCH
AF = mybir.ActivationFunctionType
ALU = mybir.AluOpType
EPS = 1e-12


def build_program(gamma: float):
    nc = bacc.Bacc("TRN2", target_bir_lowering=False, debug=False)

    x1_d = nc.dram_tensor("x1", (C, H, W), F32, kind="ExternalInput")
    x2_d = nc.dram_tensor("x2", (C, H, W), F32, kind="ExternalInput")
    wp_d = nc.dram_tensor("wpT", (C, C), F32, kind="ExternalInput")    # W_proj.T
    wm1_d = nc.dram_tensor("wm1T", (C, C), F32, kind="ExternalInput")
    wm2_d = nc.dram_tensor("wm2T", (C, C), F32, kind="ExternalInput")
    bp_d = nc.dram_tensor("bp", (C, 1), F32, kind="ExternalInput")
    bm1_d = nc.dram_tensor("bm1", (C, 1), F32, kind="ExternalInput")
    bm2_d = nc.dram_tensor("bm2", (C, 1), F32, kind="ExternalInput")
    out_d = nc.dram_tensor("out", (C, H, W), F32, kind="ExternalOutput")

    x1f = x1_d[:].rearrange("c h w -> c (h w)")
    x2f = x2_d[:].rearrange("c h w -> c (h w)")
    outf = out_d[:].rearrange("c h w -> c (h w)")

    g1, g2 = float(gamma), float(1.0 - gamma)

    with tile.TileContext(nc) as tc:
        with tc.tile_pool(name="const", bufs=1) as cpool, \
             tc.tile_pool(name="dram", bufs=1, space="DRAM") as dpool:
            # ---- persistent constants / weights ----
            ident = cpool.tile([128, 128], F32, tag="identf")
            make_identity(nc, ident[:])
            ident_r = cpool.tile([128, 128], F32R, tag="identr")
            nc.vector.tensor_copy(ident_r[:], ident[:])
            ones_f = cpool.tile([128, 128], F32, tag="onesf")
            nc.gpsimd.memset(ones_f[:], 1.0)
            ones_r = cpool.tile([128, 128], F32R, tag="onesr")
            nc.vector.tensor_copy(ones_r[:], ones_f[:])

            wts = {}
            for nm, dt_ in (("wp", wp_d), ("wm1", wm1_d), ("wm2", wm2_d)):
                for k in range(2):
                    t = cpool.tile([96, C], F32R, tag=f"{nm}{k}")
                    nc.sync.dma_start(t[:], dt_[96 * k:96 * (k + 1), :].bitcast(F32R))
                    wts[f"{nm}{k}"] = t
            for nm, dt_ in (("bp", bp_d), ("bm1", bm1_d), ("bm2", bm2_d)):
                for m in range(2):
                    t = cpool.tile([96, 1], F32, tag=f"{nm}{m}")
                    nc.sync.dma_start(t[:], dt_[96 * m:96 * (m + 1), :])
                    wts[f"{nm}{m}"] = t

            o1sp = dpool.tile([C, HW], F32, tag="o1sp")
            o2sp = dpool.tile([C, HW], F32, tag="o2sp")
            fus_sp = dpool.tile([C, HW], F32, tag="fussp")

            # ================= phase 1: projections =================
            with tc.tile_pool(name="p1x", bufs=3) as xp, \
                 tc.tile_pool(name="p1s", bufs=4) as sp, \
                 tc.tile_pool(name="p1ps", bufs=4, space="PSUM") as pp:
                for s in range(NCHUNKS):
                    sl = bass.ts(s, NCH)
                    for xf, osp in ((x1f, o1sp), (x2f, o2sp)):
                        xa = xp.tile([96, NCH], F32R, tag="xa")
                        xb = xp.tile([96, NCH], F32R, tag="xb")
                        nc.sync.dma_start(xa[:], xf[0:96, sl].bitcast(F32R))
                        nc.sync.dma_start(xb[:], xf[96:192, sl].bitcast(F32R))
                        for m in range(2):
                            ps = pp.tile([96, NCH], F32, tag="ps")
                            msl = bass.ts(m, 96)
                            nc.tensor.matmul(ps[:], wts["wp0"][:, msl], xa[:], start=True, stop=False)
                            nc.tensor.matmul(ps[:], wts["wp1"][:, msl], xb[:], start=False, stop=True)
                            st = sp.tile([96, NCH], F32, tag="st")
                            nc.scalar.activation(st[:], ps[:], AF.Identity, bias=wts[f"bp{m}"][:])
                            nc.sync.dma_start(osp[96 * m:96 * (m + 1), sl], st[:])

            # ================= phase 2: axial attention per head =================
            o1v = o1sp[:].rearrange("c (h w) -> h c w", h=H)   # [128, 192, 128] view
            o2v = o2sp[:].rearrange("c (h w) -> h c w", h=H)
            fusv = fus_sp[:].rearrange("c (h w) -> h c w", h=H)

            with tc.tile_pool(name="nat", bufs=2) as natp, \
                 tc.tile_pool(name="trn", bufs=1) as trnp, \
                 tc.tile_pool(name="qq", bufs=1) as qp, \
                 tc.tile_pool(name="fus", bufs=2) as fusp, \
                 tc.tile_pool(name="sm", bufs=2) as smp, \
                 tc.tile_pool(name="tiny", bufs=2) as tp, \
                 tc.tile_pool(name="junk", bufs=1) as jp, \
                 tc.tile_pool(name="pst", bufs=2, space="PSUM") as pst, \
                 tc.tile_pool(name="psg", bufs=2, space="PSUM") as psg, \
                 tc.tile_pool(name="psb", bufs=2, space="PSUM") as psb, \
                 tc.tile_pool(name="psav", bufs=2, space="PSUM") as psav:
                for g in range(HEADS):
                    hsl = slice(CH * g, CH * (g + 1))
                    o1n = natp.tile([H, CH, W], F32R, tag="o1n")
                    o2n = natp.tile([H, CH, W], F32R, tag="o2n")
                    nc.sync.dma_start(o1n[:], o1v[:, hsl, :].bitcast(F32R))
                    nc.sync.dma_start(o2n[:], o2v[:, hsl, :].bitcast(F32R))

                    # transposed per-channel planes: o1t/o2t [w, c, h]
                    o1t = trnp.tile([W, CH, H], F32R, tag="o1t")
                    o2t = trnp.tile([W, CH, H], F32R, tag="o2t")
                    for (src, dst) in ((o1n, o1t), (o2n, o2t)):
                        for c in range(CH):
                            pt_ = pst.tile([128, 128], F32, tag="pt")
                            nc.tensor.matmul(pt_[:].bitcast(F32R), src[:, c, :], ident_r[:], is_transpose=True)
                            eng = nc.vector if c % 2 == 0 else nc.scalar
                            if eng is nc.vector:
                                nc.vector.tensor_copy(dst[:, c, :], pt_[:])
                            else:
                                nc.scalar.copy(dst[:, c, :], pt_[:])

                    # ---- norms: nh1,nh2 from natural planes; nw1,nw2 from transposed
                    junk = jp.tile([128, CH * 128], F32, tag="junk")
                    rn = {}
                    for nm, src in (("nh1", o1n), ("nh2", o2n), ("nw1", o1t), ("nw2", o2t)):
                        sq = tp.tile([128, 1], F32, tag=f"sq_{nm}")
                        v = src[:].rearrange("p a b -> p (a b)").bitcast(F32)
                        nc.vector.scalar_tensor_tensor(junk[:], v, 1.0, v, op0=ALU.mult, op1=ALU.mult, accum_out=sq[:])
                        n_ = tp.tile([128, 1], F32, tag=f"n_{nm}")
                        nc.scalar.sqrt(n_[:], sq[:])
                        nc.vector.tensor_scalar_max(n_[:], n_[:], EPS)
                        r_ = tp.tile([128, 1], F32, tag=f"r_{nm}")
                        nc.vector.reciprocal(r_[:], n_[:])
                        rn[nm] = r_

                    # scaled variants
                    sc = {}
                    for nm, src, f in (("q1s", "nh2", g1), ("q2s", "nw1", g2),
                                       ("h100", "nh2", 100.0), ("w100", "nw1", 100.0)):
                        t = tp.tile([128, 1], F32, tag=f"sc_{nm}")
                        nc.scalar.mul(t[:], rn[src][:], f)
                        sc[nm] = t

                    # ---- Grams ----
                    psSw = psg.tile([128, 128], F32, tag="gram")
                    for c in range(CH):
                        nc.tensor.matmul(psSw[:], o1n[:, c, :], o2n[:, c, :], start=(c == 0), stop=(c == CH - 1))
                    psSh = psg.tile([128, 128], F32, tag="gram")
                    for c in range(CH):
                        nc.tensor.matmul(psSh[:], o2t[:, c, :], o1t[:, c, :], start=(c == 0), stop=(c == CH - 1))

                    # ---- softmax chains (w then h). P-transposed lhsT kept in SBUF.
                    PTs = {}
                    dens = {}
                    for side, psS, rq100, rkey, gscale in (
                        ("w", psSw, sc["w100"], rn["nw2"], g2),
                        ("h", psSh, sc["h100"], rn["nh1"], g1),
                    ):
                        D = smp.tile([128, 128], F32R, tag="D")
                        nc.vector.tensor_scalar_mul(D[:], ident_r[:], rkey[:])
                        psB = psb.tile([128, 128], F32, tag="psB")
                        nc.tensor.matmul(psB[:], ones_r[:], D[:])
                        Bs = smp.tile([128, 128], F32, tag="Bs")
                        nc.scalar.copy(Bs[:], psB[:])
                        Sp = smp.tile([128, 128], F32, tag="Sp")
                        nc.vector.scalar_tensor_tensor(Sp[:], psS[:], rq100[:], Bs[:], op0=ALU.mult, op1=ALU.mult)
                        nmax = tp.tile([128, 1], F32, tag="nmax")
                        nc.vector.reduce_max(nmax[:], Sp[:], axis=mybir.AxisListType.X, negate=True)
                        eS = smp.tile([128, 128], F32, tag="eS")
                        den = tp.tile([128, 1], F32, tag="den")
                        nc.scalar.activation(eS[:], Sp[:], AF.Exp, bias=nmax[:], accum_out=den[:])
                        rden = tp.tile([128, 1], F32, tag="rden")
                        nc.vector.reciprocal(rden[:], den[:])
                        rdg = tp.tile([128, 1], F32, tag="rdg")
                        nc.scalar.mul(rdg[:], rden[:], gscale)
                        eSs = smp.tile([128, 128], F32R, tag="eSs")
                        nc.vector.tensor_scalar_mul(eSs[:], eS[:], rdg[:])
                        psT = psb.tile([128, 128], F32, tag="psB")
                        nc.tensor.matmul(psT[:].bitcast(F32R), eSs[:], ident_r[:], is_transpose=True)
                        PT = smp.tile([128, 128], F32R, tag=f"PT{side}")
                        nc.vector.tensor_copy(PT[:], psT[:])
                        PTs[side] = PT
                        dens[side] = rden

                    # ---- q terms (scaled by gamma factors) ----
                    q2g = qp.tile([128, CH * 128], F32R, tag="q2g")   # also becomes fus4
                    nc.scalar.activation(q2g[:], o1t[:].rearrange("p a b -> p (a b)").bitcast(F32),
                                         AF.Copy, scale=sc["q2s"][:])
                    q1g = qp.tile([128, CH * 128], F32, tag="q1g")
                    nc.scalar.activation(q1g[:], o2n[:].rearrange("p a b -> p (a b)").bitcast(F32),
                                         AF.Copy, scale=sc["q1s"][:])

                    # ---- w-attention AV into q2g (in place): fus4 = (1-g)*out4  [w, c, h]
                    for t6 in range(6):
                        psO = psav.tile([128, 512], F32, tag="av")
                        csl = slice(4 * t6, 4 * (t6 + 1))
                        nc.tensor.matmul(psO[:], PTs["w"][:], o2t[:, csl, :])
                        qsl = bass.ts(t6, 512)
                        nc.vector.tensor_add(q2g[:, qsl], psO[:], q2g[:, qsl].bitcast(F32))

                    # ---- h-attention AV + transposed fus4 accumulation + epilogue
                    fus_h = fusp.tile([H, CH, W], F32, tag="fush")
                    for t6 in range(6):
                        psO = psav.tile([128, 512], F32, tag="av")
                        csl = slice(4 * t6, 4 * (t6 + 1))
                        nc.tensor.matmul(psO[:], PTs["h"][:], o1n[:, csl, :])
                        for c4 in range(4):
                            cc = 4 * t6 + c4
                            plane = q2g[:, 128 * cc:128 * (cc + 1)]
                            nc.tensor.matmul(psO[:, 128 * c4:128 * (c4 + 1)].bitcast(F32R), plane, ident_r[:],
                                             is_transpose=True, start=False, stop=True)
                        qsl = bass.ts(t6, 512)
                        nc.vector.tensor_add(fus_h[:].rearrange("p a b -> p (a b)")[:, qsl], psO[:], q1g[:, qsl])

                    nc.sync.dma_start(fusv[:, hsl, :], fus_h[:])

            # ================= phase 3: final conv chain + residual =================
            fusf = fus_sp[:]
            with tc.tile_pool(name="p3f", bufs=3) as fp, \
                 tc.tile_pool(name="p3t", bufs=3) as tp3, \
                 tc.tile_pool(name="p3ps", bufs=4, space="PSUM") as pp3:
                for s in range(NCHUNKS):
                    sl = bass.ts(s, NCH)
                    fA = fp.tile([96, NCH], F32R, tag="fA")
                    fB = fp.tile([96, NCH], F32R, tag="fB")
                    nc.sync.dma_start(fA[:], fusf[0:96, sl].bitcast(F32R))
                    nc.sync.dma_start(fB[:], fusf[96:192, sl].bitcast(F32R))
                    res = []
                    for m in range(2):
                        msl_d = slice(96 * m, 96 * (m + 1))
                        r1 = fp.tile([96, NCH], F32, tag="r1")
                        r2 = fp.tile([96, NCH], F32, tag="r2")
                        nc.sync.dma_start(r1[:], x1f[msl_d, sl])
                        nc.sync.dma_start(r2[:], x2f[msl_d, sl])
                        res.append((r1, r2))
                    t5 = []
                    for m in range(2):
                        ps = pp3.tile([96, NCH], F32, tag="ps3")
                        msl = bass.ts(m, 96)
                        nc.tensor.matmul(ps[:], wts["wp0"][:, msl], fA[:], start=True, stop=False)
                        nc.tensor.matmul(ps[:], wts["wp1"][:, msl], fB[:], start=False, stop=True)
                        t = tp3.tile([96, NCH], F32R, tag="t5")
                        nc.scalar.activation(t[:], ps[:], AF.Identity, bias=wts[f"bp{m}"][:])
                        t5.append(t)
                    t6 = []
                    for m in range(2):
                        ps = pp3.tile([96, NCH], F32, tag="ps3")
                        msl = bass.ts(m, 96)
                        nc.tensor.matmul(ps[:], wts["wm10"][:, msl], t5[0][:], start=True, stop=False)
                        nc.tensor.matmul(ps[:], wts["wm11"][:, msl], t5[1][:], start=False, stop=True)
                        t = tp3.tile([96, NCH], F32R, tag="t6")
                        nc.scalar.activation(t[:], ps[:], AF.Gelu, bias=wts[f"bm1{m}"][:])
                        t6.append(t)
                    for m in range(2):
                        ps = pp3.tile([96, NCH], F32, tag="ps3")
                        msl = bass.ts(m, 96)
                        nc.tensor.matmul(ps[:], wts["wm20"][:, msl], t6[0][:], start=True, stop=False)
                        nc.tensor.matmul(ps[:], wts["wm21"][:, msl], t6[1][:], start=False, stop=True)
                        s1 = tp3.tile([96, NCH], F32, tag="s1")
                        r1, r2 = res[m]
                        nc.vector.scalar_tensor_tensor(s1[:], ps[:], wts[f"bm2{m}"][:], r1[:], op0=ALU.add, op1=ALU.add)
                        s2 = tp3.tile([96, NCH], F32, tag="s2")
                        nc.vector.tensor_add(s2[:], s1[:], r2[:])
                        nc.sync.dma_start(outf[96 * m:96 * (m + 1), sl], s2[:])

    nc.compile()
    return nc


_CACHE = {}


def _get_program(gamma: float):
    key = round(float(gamma), 9)
    if key not in _CACHE:
        _CACHE[key] = build_program(key)
    return _CACHE[key]


def kernel(x1, x2, W_proj, b_proj, gate, pos_bias_h, pos_bias_w, W_m1, b_m1, W_m2, b_m2):
    x1 = np.asarray(x1, dtype=np.float32)
    x2 = np.asarray(x2, dtype=np.float32)
    gamma = float(1.0 / (1.0 + np.exp(-np.float32(np.asarray(gate).reshape(-1)[0]))))
    nc = _get_program(gamma)

    common = {
        "wpT": np.ascontiguousarray(np.asarray(W_proj, np.float32).T),
        "wm1T": np.ascontiguousarray(np.asarray(W_m1, np.float32).T),
        "wm2T": np.ascontiguousarray(np.asarray(W_m2, np.float32).T),
        "bp": np.asarray(b_proj, np.float32).reshape(C, 1),
        "bm1": np.asarray(b_m1, np.float32).reshape(C, 1),
        "bm2": np.asarray(b_m2, np.float32).reshape(C, 1),
    }
    B = x1.shape[0]
    in_maps = [dict(common, x1=np.ascontiguousarray(x1[b]), x2=np.ascontiguousarray(x2[b]))
               for b in range(B)]
    res = run_bass_kernel_spmd(nc, in_maps, core_ids=list(range(B)))
    return np.stack([res.results[b]["out"] for b in range(B)], axis=0)
